# revision 7
# baseline (speedup 1.0000x reference)
"""KNN-impute kernel (nn_CalcImpute) for Trainium2, 8 NeuronCores.

Computation (see reference): for each of 8192 receiver rows, find the 16
smallest entries of a 50000-wide distance row (ties -> lowest column index,
matching jax.lax.top_k), gather fit_X_col at those columns, and output the
mean of the valid (mask==0) donor values (0 if none valid).

Sharding: pure data parallel over rows; each of the 8 cores gets 1024 rows.
fit/mask-derived tables are tiny and replicated.

Device algorithm per 128-row tile (rows live in partitions):
  P1  stream the 50000 columns in 8 panels of 6250.  ACT casts each panel
      to *negated bf16* (Copy, scale=-1), deinterleaved into seg-halves so
      every later bf16 operand is 4B-aligned.  DVE folds each 50-segment
      with a 6-level pairwise tensor_tensor MAX tree (bf16 runs 2x where
      alignment allows) -> nsm = negated bf16 segment minima [P, 1000].
      This takes the full-data pass off the 1x-locked f32 tensor_reduce
      path and splits it DVE/ACT.
  P2  3 rounds of max8/max_index (+match_replace on rounds 1-2) give the
      24 segments with the smallest bf16 minima; the first KSEG=20 are
      gathered (bf16 rounding can reorder near-ties, so 4 segments of
      margin over the 16 needed; the 20th value feeds the coverage flag).
  P3  indirect-DMA gather of those 20 segments per row: raw f32 distances
      (20x50) from DRAM plus matching [G; V] table slices in bf16
      (G = fitX * valid, V = valid, precomputed on host).
  P4  exact top-16 on the gathered f32 candidates: negate (ACT); 2x
      (max8+match_replace) + 3rd max8 for the 17th value; selection mask
      = (orig != replaced); numerator/denominator via bf16 multiply +
      ACT Copy-with-accum; res = num/den (den==0 -> den=1).

Flags (host recompute of flagged rows gives exactness):
  - boundary tie: 17th candidate value == 16th.
  - bf16 coverage: some ungathered segment's true f32 min could undercut
    the 16th candidate.  Ungathered true mins are >= (20th bf16 segmin) *
    (1 - 2^-7), so flag when that bound <= v16.  Both checks run on the
    negated scale.
"""

import os
import sys

for _p in ("/opt/trn_rl_repo", "/root/.axon_site/_ro/trn_rl_repo"):
    if os.path.isdir(_p) and _p not in sys.path:
        sys.path.insert(0, _p)

import numpy as np

import concourse.bass as bass
import concourse.bacc as bacc_mod
import concourse.mybir as mybir
import concourse.tile as tile
from concourse.bass_utils import run_bass_kernel_spmd

N_CORES = 8
R_TOTAL = 8192
N = 50000
P = 128              # SBUF partitions
S = 50               # segment size for the min prefilter
NSEG = N // S        # 1000 segments per row
PC = 6250            # panel columns streamed per DMA
NPAN = N // PC       # 8 panels
NSEG_P = PC // S     # 125 segments per panel
KSEG = 20            # candidate segments gathered per row
NIDX = 24            # segment indices extracted (3 max8 rounds)
CAND = KSEG * S      # 1000 candidate values per row
NEG_BIG = -3.0e38    # replacement sentinel on the negated scale
COVER = 0.9921875    # 1 - 2^-7: bf16 rounding safety factor
F32 = mybir.dt.float32
BF16 = mybir.dt.bfloat16
U32 = mybir.dt.uint32


def build_bass(rows: int, repeat: int = 1):
    """Bass program for one core processing `rows` rows (multiple of 128).

    repeat>1 re-runs the whole pipeline (for slope-based benchmarking).
    """
    assert rows % P == 0
    nt = rows // P

    nc = bacc_mod.Bacc()
    dist = nc.dram_tensor("dist", [rows, N], F32, kind="ExternalInput")
    gv = nc.dram_tensor("gv", [NSEG, 2 * S], BF16, kind="ExternalInput")
    out_res = nc.dram_tensor("res", [P, nt], F32, kind="ExternalOutput")
    out_flag = nc.dram_tensor("flag", [P, nt], F32, kind="ExternalOutput")

    # flat views for indirect gathers (offset must be 0)
    dist_flat = dist[:, :].rearrange("r (s e) -> (r s) e", e=S)
    gv_flat = gv[:, :]

    with tile.TileContext(nc) as tc:
        with (
            tc.tile_pool(name="panels", bufs=4) as pan_pool,
            tc.tile_pool(name="casts", bufs=2) as cast_pool,
            tc.tile_pool(name="tree", bufs=1) as tree_pool,
            tc.tile_pool(name="segs", bufs=2) as seg_pool,
            tc.tile_pool(name="small", bufs=2) as small_pool,
            tc.tile_pool(name="cands", bufs=2) as cand_pool,
            tc.tile_pool(name="scratch", bufs=1) as scr_pool,
            tc.tile_pool(name="persist", bufs=1) as persist_pool,
        ):
            res_sb = persist_pool.tile([P, nt], F32)
            flag_sb = persist_pool.tile([P, nt], F32)

            def emit_tile(rt):
                """P1 panels+segmin tree, P2 top segments, P3 gathers."""
                nsm = seg_pool.tile([P, NSEG], BF16, tag="nsm")
                for pan in range(NPAN):
                    x = pan_pool.tile([P, PC], F32, tag="panel")
                    nc.sync.dma_start(
                        out=x,
                        in_=dist[rt * P:(rt + 1) * P, pan * PC:(pan + 1) * PC],
                    )
                    x3 = x.rearrange("p (s e) -> p s e", e=S)
                    # negated bf16 casts, deinterleaved into seg halves so
                    # all bf16 tree operands are 4B-aligned
                    xa = cast_pool.tile([P, NSEG_P, 25], BF16, tag="xa")
                    xb = cast_pool.tile([P, NSEG_P, 25], BF16, tag="xb")
                    nc.scalar.activation(
                        out=xa, in_=x3[:, :, 0:25],
                        func=mybir.ActivationFunctionType.Copy, scale=-1.0)
                    nc.scalar.activation(
                        out=xb, in_=x3[:, :, 25:50],
                        func=mybir.ActivationFunctionType.Copy, scale=-1.0)
                    # 6-level pairwise MAX tree (negated mins). Overlapping
                    # pairings cover odd widths; max is idempotent.
                    t1 = tree_pool.tile([P, NSEG_P, 25], BF16, tag="t1")
                    t2 = tree_pool.tile([P, NSEG_P, 13], BF16, tag="t2")
                    t3 = tree_pool.tile([P, NSEG_P, 7], BF16, tag="t3")
                    t4 = tree_pool.tile([P, NSEG_P, 4], BF16, tag="t4")
                    t5 = tree_pool.tile([P, NSEG_P, 2], BF16, tag="t5")
                    mx = mybir.AluOpType.max
                    tt = nc.vector.tensor_tensor
                    tt(out=t1, in0=xa, in1=xb, op=mx)
                    tt(out=t2, in0=t1[:, :, 0:13], in1=t1[:, :, 12:25], op=mx)
                    tt(out=t3, in0=t2[:, :, 0:7], in1=t2[:, :, 6:13], op=mx)
                    tt(out=t4, in0=t3[:, :, 0:4], in1=t3[:, :, 3:7], op=mx)
                    tt(out=t5, in0=t4[:, :, 0:2], in1=t4[:, :, 2:4], op=mx)
                    nsm3 = nsm.rearrange("p (s e) -> p s e", e=1)
                    tt(out=nsm3[:, pan * NSEG_P:(pan + 1) * NSEG_P, :],
                       in0=t5[:, :, 0:1], in1=t5[:, :, 1:2], op=mx)

                # P2: top segments by negated bf16 seg-min (descending max8)
                segidx = small_pool.tile([P, NIDX], U32, tag="segidx")
                v_seg = small_pool.tile([P, 3, 8], BF16, tag="v_seg")
                for rnd in range(3):
                    v8 = v_seg[:, rnd, :]
                    nc.vector.max(out=v8, in_=nsm)
                    nc.vector.max_index(
                        out=segidx[:, rnd * 8:(rnd + 1) * 8],
                        in_max=v8, in_values=nsm)
                    if rnd < 2:
                        nc.vector.match_replace(
                            out=nsm, in_to_replace=v8, in_values=nsm,
                            imm_value=NEG_BIG)
                # f32 copy of the 20th seg-min for the coverage flag
                v20f = small_pool.tile([P, 1], F32, tag="v20f")
                nc.scalar.activation(
                    out=v20f, in_=v_seg[:, 2, 3:4],
                    func=mybir.ActivationFunctionType.Copy)

                # P3: gather candidate segments + G/V slices.
                rowbase = small_pool.tile([P, 1], U32, tag="rowbase")
                nc.gpsimd.iota(rowbase, pattern=[[0, 1]],
                               base=rt * P * NSEG, channel_multiplier=NSEG)
                off_dist = small_pool.tile([P, KSEG], U32, tag="off_dist")
                nc.vector.tensor_tensor(
                    out=off_dist, in0=segidx[:, 0:KSEG],
                    in1=rowbase.to_broadcast([P, KSEG]),
                    op=mybir.AluOpType.add)
                # HW SWDGE indirect gather is only reliable with one offset
                # per partition, so issue one gather per candidate column.
                cand = cand_pool.tile([P, KSEG, S], F32, tag="cand")
                gvc = cand_pool.tile([P, KSEG, 2 * S], BF16, tag="gvc")
                for t in range(KSEG):
                    nc.gpsimd.indirect_dma_start(
                        out=cand[:, t, :], out_offset=None,
                        in_=dist_flat,
                        in_offset=bass.IndirectOffsetOnAxis(
                            ap=off_dist[:, t:t + 1], axis=0),
                    )
                    nc.gpsimd.indirect_dma_start(
                        out=gvc[:, t, :], out_offset=None,
                        in_=gv_flat,
                        in_offset=bass.IndirectOffsetOnAxis(
                            ap=segidx[:, t:t + 1], axis=0),
                    )
                return dict(rt=rt, cand=cand, gvc=gvc, v20f=v20f)

            def emit_p4(st):
                """Exact top-16 + weighted mean for a tile whose gathers
                completed during the next tile's panel streaming."""
                rt, cand, gvc, v20f = st["rt"], st["cand"], st["gvc"], st["v20f"]
                ncand = scr_pool.tile([P, CAND], F32, tag="ncand")
                ncandb = scr_pool.tile([P, CAND], F32, tag="ncandb")
                sel = scr_pool.tile([P, CAND], BF16, tag="sel")
                junk = scr_pool.tile([P, CAND], BF16, tag="junk")
                junk2 = scr_pool.tile([P, CAND], BF16, tag="junk2")
                v_c = small_pool.tile([P, 3, 8], F32, tag="v_c")
                acc = small_pool.tile([P, 8], F32, tag="acc")
                num, den = acc[:, 0:1], acc[:, 1:2]
                sel3 = sel.rearrange("p (a b) -> p a b", b=S)
                junk3 = junk.rearrange("p (a b) -> p a b", b=S)

                nc.scalar.mul(ncand, cand.rearrange("p a b -> p (a b)"), -1.0)
                nc.vector.max(out=v_c[:, 0, :], in_=ncand)
                nc.vector.match_replace(
                    out=ncandb, in_to_replace=v_c[:, 0, :],
                    in_values=ncand, imm_value=NEG_BIG)
                nc.vector.max(out=v_c[:, 1, :], in_=ncandb)
                nc.vector.match_replace(
                    out=ncandb, in_to_replace=v_c[:, 1, :],
                    in_values=ncandb, imm_value=NEG_BIG)
                nc.vector.max(out=v_c[:, 2, :], in_=ncandb)
                nc.vector.tensor_tensor(
                    out=sel, in0=ncand, in1=ncandb,
                    op=mybir.AluOpType.not_equal)
                # (tensor_tensor_reduce crashes the exec unit on this HW;
                # multiply on DVE, sum via ACT Copy-with-accum)
                nc.vector.tensor_tensor(out=junk3, in0=sel3,
                                        in1=gvc[:, :, 0:S],
                                        op=mybir.AluOpType.mult)
                nc.scalar.activation(
                    out=junk2, in_=junk,
                    func=mybir.ActivationFunctionType.Copy,
                    accum_out=num)
                nc.vector.tensor_tensor(out=junk3, in0=sel3,
                                        in1=gvc[:, :, S:2 * S],
                                        op=mybir.AluOpType.mult)
                nc.scalar.activation(
                    out=junk2, in_=junk,
                    func=mybir.ActivationFunctionType.Copy,
                    accum_out=den)
                # denp = den + (den == 0)
                denp, recip = acc[:, 3:4], acc[:, 4:5]
                nc.vector.scalar_tensor_tensor(
                    out=denp, in0=den, scalar=0.0, in1=den,
                    op0=mybir.AluOpType.is_equal,
                    op1=mybir.AluOpType.add)
                nc.vector.reciprocal(recip, denp)
                nc.vector.tensor_mul(res_sb[:, rt:rt + 1], num, recip)
                # flag = max(v17_cand, COVER * v20_seg) >= v16_cand
                # (negated scale; nsm values are negative, so * COVER moves
                # the bound toward zero = conservative)
                flagtmp = acc[:, 5:6]
                nc.vector.scalar_tensor_tensor(
                    out=flagtmp, in0=v20f, scalar=COVER,
                    in1=v_c[:, 2, 0:1],
                    op0=mybir.AluOpType.mult, op1=mybir.AluOpType.max)
                nc.vector.tensor_tensor(
                    out=flag_sb[:, rt:rt + 1], in0=flagtmp,
                    in1=v_c[:, 1, 7:8], op=mybir.AluOpType.is_ge)

            # software pipeline: P4 of tile t is emitted after tile t+1's
            # panel loop + P2 + P3, so its gathers have a full tile of
            # panel streaming to land behind.
            pending = None
            for rt in [t for _ in range(repeat) for t in range(nt)]:
                st = emit_tile(rt)
                if pending is not None:
                    emit_p4(pending)
                pending = st
            emit_p4(pending)

            nc.sync.dma_start(out=out_res[:, :], in_=res_sb)
            nc.sync.dma_start(out=out_flag[:, :], in_=flag_sb)

    nc.compile()
    return nc


def _host_reference_rows(dist_rows: np.ndarray, fit: np.ndarray,
                         mask: np.ndarray, k: int) -> np.ndarray:
    """Exact recompute (jax.lax.top_k tie semantics) for flagged rows."""
    out = np.empty(dist_rows.shape[0], dtype=np.float32)
    valid = (1 - mask).astype(np.float32)
    for i, row in enumerate(dist_rows):
        r = np.nan_to_num(row, nan=1e10)
        idx = np.argsort(r, kind="stable")[:k]
        w = valid[idx]
        ws = np.float32(w.sum(dtype=np.float32))
        div = ws if ws != 0 else np.float32(1.0)
        num = np.float32((fit[idx].astype(np.float32) * w).sum(dtype=np.float32))
        out[i] = num / div
    return out


def _prep_tables(fit_X_col: np.ndarray, mask_fit_X_col: np.ndarray):
    import ml_dtypes
    valid = (1 - mask_fit_X_col).astype(np.float32)
    g = fit_X_col.astype(np.float32) * valid
    gv_tab = np.empty((NSEG, 2, S), dtype=ml_dtypes.bfloat16)
    gv_tab[:, 0, :] = g.reshape(NSEG, S).astype(ml_dtypes.bfloat16)
    gv_tab[:, 1, :] = valid.reshape(NSEG, S).astype(ml_dtypes.bfloat16)
    return gv_tab.reshape(NSEG, 2 * S)


def kernel(dist_pot_donors, n_neighbors, fit_X_col, mask_fit_X_col,
           _trace=False, _tmpdir=None):
    dist = np.ascontiguousarray(np.asarray(dist_pot_donors, dtype=np.float32))
    fit = np.asarray(fit_X_col, dtype=np.float32)
    mask = np.asarray(mask_fit_X_col)
    k = int(np.asarray(n_neighbors))
    assert dist.shape == (R_TOTAL, N) and k == 16, (dist.shape, k)

    gv_tab = _prep_tables(fit, mask)
    rows = R_TOTAL // N_CORES
    nt = rows // P

    nc = build_bass(rows)
    in_maps = [
        {"dist": dist[c * rows:(c + 1) * rows], "gv": gv_tab}
        for c in range(N_CORES)
    ]
    kw = {}
    if _trace:
        kw.update(trace=True, tmpdir=_tmpdir)
    br = run_bass_kernel_spmd(nc, in_maps, core_ids=list(range(N_CORES)), **kw)

    out = np.empty(R_TOTAL, dtype=np.float32)
    flags = np.empty(R_TOTAL, dtype=bool)
    for c, r in enumerate(br.results):
        # res[p, t] holds row c*rows + t*128 + p
        out[c * rows:(c + 1) * rows] = r["res"].T.reshape(rows)
        flags[c * rows:(c + 1) * rows] = r["flag"].T.reshape(rows) != 0

    n_flagged = int(flags.sum())
    if n_flagged:
        out[flags] = _host_reference_rows(dist[flags], fit, mask, k)
    kernel._last = {"exec_time_ns": br.exec_time_ns,
                    "mean_exec_time_ns": br.mean_exec_time_ns,
                    "n_flagged": n_flagged,
                    "trace": br.instructions_and_trace}
    return out
